# revision 13
# baseline (speedup 1.0000x reference)
"""Trainium2 Bass kernel for MQA causal attention with null token.

Problem (reference.py):
  b=4, n=2048, dim=1024, HEADS=16, DIM_HEAD=64
  q  = (x @ Wq).reshape(b,n,16,64).transpose -> [b,h,n,64] * 64**-0.5
  kv = x @ Wkv -> [b,n,64]; prepend null -> [b,2049,64]
  sim = q @ kv^T  (causal: query i sees kv cols 0..i+1)
  out = softmax(sim) @ kv -> concat heads -> @ Wout

Sharding: 8 cores = batch(4) x head-half(2). Each core handles one batch
element and 8 heads, producing a partial out-projection; host adds the two
half-head partials per batch.

Device algorithm (per core), matmuls f16 inputs w/ fp32 PSUM accumulate,
emission interleaved qb-outer so projection / out-projection matmuls fill
PE gaps during ACT(exp)-bound attention stretches:
  per q-block (512 queries): project QT2 (head pairs on partitions) + KV
  block; build KV_aug chunks ([128,65] parity0 / [128,128] parity1 with
  output partitions 63..127); scores TRANSPOSED (keys on partitions) with
  diagonal chunks narrowed to the causally-visible range; exp on ACT
  (PSUM->SBUF f16, the +1-edge key's score rides in the diagonal tile);
  causal mask multiply on DVE; PV accumulates [*,512] with the softmax
  denominator as an extra output row; normalize = DVE reciprocal + DMA
  partition-broadcast + DVE multiply into AT; out-projection per 128-token
  chunk contract AT^T @ Wout -> fp32 -> HBM.
"""

import sys

for _p in ("/opt/trn_rl_repo",):
    if _p not in sys.path:
        sys.path.insert(0, _p)

import numpy as np

HEADS = 16
DH = 64
B = 4
N = 2048
DIM = 1024
NQB = 4          # q blocks of 512 per head
QB = 512
KTOT = N + 1     # 2049 kv positions (null at 0)

_PROGRAM_CACHE = {}


def _build_program(reps=0, expst_bufs=6, ot_bufs=2, pj_bufs=2, st_bufs=2,
                   ost_bufs=3, bcast_via="pool"):
    import concourse.bacc as bacc
    import concourse.tile as tile
    import concourse.mybir as mybir
    import concourse.bass as _bass

    f16 = mybir.dt.float16
    f32 = mybir.dt.float32
    EXP = mybir.ActivationFunctionType.Exp

    nc = bacc.Bacc("TRN2", debug=False, num_devices=8)

    xt_d = nc.dram_tensor("xt", [DIM, N], f16, kind="ExternalInput").ap()
    wq_d = nc.dram_tensor("wq", [DIM, 512], f16, kind="ExternalInput").ap()
    wkv2_d = nc.dram_tensor("wkv2", [DIM, 128], f16, kind="ExternalInput").ap()
    nullkv2_d = nc.dram_tensor("nullkv2", [128, 1], f16, kind="ExternalInput").ap()
    wout_d = nc.dram_tensor("wout", [512, DIM], f16, kind="ExternalInput").ap()
    masks_d = nc.dram_tensor("masks", [128, 4 * QB], f16, kind="ExternalInput").ap()
    ident_d = nc.dram_tensor("ident", [128, 128], f16, kind="ExternalInput").ap()
    out_d = nc.dram_tensor("out", [N, DIM], f32, kind="ExternalOutput").ap()

    # diagonal chunk t: visible cols j >= j0; (j0, width), col offset in tile
    DIAG = []
    off = [0, 0]
    for t in range(4):
        j0 = 0 if t == 0 else 128 * t - 1
        w = QB - j0
        grp = t // 2
        DIAG.append((t, j0, w, grp, off[grp]))
        off[grp] += w
    GW = (off[0], off[1])          # (897, 386)
    EDGE_OFF = off[1]              # edge score column in the grp-1 tile

    with tile.TileContext(nc) as tc:
        from contextlib import ExitStack

        with ExitStack() as ctx:
            consts = ctx.enter_context(tc.tile_pool(name="consts", bufs=1))
            work = ctx.enter_context(tc.tile_pool(name="work", bufs=expst_bufs))
            ostp = ctx.enter_context(tc.tile_pool(name="ostp", bufs=ost_bufs))
            small = ctx.enter_context(tc.tile_pool(name="small", bufs=3))
            # PSUM budget (8 banks): st 2x2 + ot 2x1 + pj 2x1 = 8
            st_ps = ctx.enter_context(tc.tile_pool(
                name="st_ps", bufs=st_bufs, space="PSUM"))
            ot_ps = ctx.enter_context(tc.tile_pool(
                name="ot_ps", bufs=ot_bufs, space="PSUM"))
            pj_ps = ctx.enter_context(tc.tile_pool(
                name="pj_ps", bufs=pj_bufs, space="PSUM"))

            # ---- persistent SBUF tiles ----
            xt_sb = consts.tile([128, 8, N], f16, tag="xt")
            wq_sb = consts.tile([128, 8, 512], f16, tag="wq")
            wkv2_sb = consts.tile([128, 8, 128], f16, tag="wkv2")
            wout_sb = consts.tile([128, 4, DIM], f16, tag="wout")
            masks_sb = consts.tile([128, 4 * QB], f16, tag="masks")
            ident_sb = consts.tile([128, 128], f16, tag="ident")
            kvt2_sb = consts.tile([128, KTOT], f16, tag="kvt2")
            kvaug_sb = consts.tile([128, 17 * 65], f16, tag="kvaug")
            kvaug2_sb = consts.tile([128, 17 * 128], f16, tag="kvaug2")
            qt2_sb = consts.tile([128, 4, N], f16, tag="qt2")
            at_sb = consts.tile([128, 4, N], f16, tag="at")
            ones_sb = consts.tile([128, 64], f16, tag="ones")

            # one-time setup (outside the timing loop)
            nc.vector.memset(kvaug2_sb, 0.0)
            nc.vector.memset(ones_sb, 1.0)

            xt_r = xt_d.rearrange("(d p) t -> p d t", p=128)

            def emit_inputs_qb(qb):
                qs = slice(qb * QB, (qb + 1) * QB)
                nc.sync.dma_start(out=xt_sb[:, :, qs], in_=xt_r[:, :, qs])

            wq_r = wq_d.rearrange("(d p) m -> p d m", p=128)

            def emit_inputs_weights_early():
                nc.sync.dma_start(
                    out=wkv2_sb, in_=wkv2_d.rearrange("(d p) m -> p d m", p=128))

            def emit_inputs_weights_late():
                nc.sync.dma_start(out=masks_sb, in_=masks_d)
                nc.sync.dma_start(
                    out=wout_sb, in_=wout_d.rearrange("(f p) o -> p f o", p=128))

            def emit_proj_kv(qb):
                qs = slice(qb * QB, (qb + 1) * QB)
                kp = pj_ps.tile([128, 512], f32, tag="pj")
                for d in range(8):
                    nc.tensor.matmul(
                        kp[:, 0:512],
                        lhsT=wkv2_sb[:, d, :],
                        rhs=xt_sb[:, d, qs],
                        start=(d == 0),
                        stop=(d == 7),
                    )
                nc.vector.tensor_copy(kvt2_sb[:, 1 + qb * 512:513 + qb * 512],
                                   kp[:, 0:512])

            def emit_proj_q(qb, pair):
                qs = slice(qb * QB, (qb + 1) * QB)
                qp = pj_ps.tile([128, 512], f32, tag="pj")
                for d in range(8):
                    nc.tensor.matmul(
                        qp[:, 0:512],
                        lhsT=wq_sb[:, d, pair * 128:(pair + 1) * 128],
                        rhs=xt_sb[:, d, qs],
                        start=(d == 0),
                        stop=(d == 7),
                    )
                nc.vector.tensor_copy(qt2_sb[:, pair, qs], qp[:, 0:512])

            def emit_proj(qb):
                emit_proj_kv(qb)
                for pair in range(4):
                    emit_proj_q(qb, pair)

            def emit_kvaug(qb):
                for c in range(4 * qb, 4 * qb + 4):
                    tp = pj_ps.tile([128, 64], f16, tag="pj")
                    nc.tensor.transpose(
                        tp, kvt2_sb[0:64, c * 128:(c + 1) * 128],
                        ident_sb[0:64, 0:64]
                    )
                    nc.vector.tensor_copy(kvaug_sb[:, c * 65:c * 65 + 64], tp)
                    nc.vector.memset(kvaug_sb[:, c * 65 + 64:c * 65 + 65], 1.0)
                    nc.vector.tensor_copy(
                        kvaug2_sb[:, c * 128 + 64:c * 128 + 128], tp)
                    nc.vector.memset(
                        kvaug2_sb[:, c * 128 + 32:c * 128 + 33], 1.0)
                cE = 4 * qb + 4
                kE = 128 * cE
                tpe = pj_ps.tile([128, 64], f16, tag="pj")
                nc.tensor.transpose(
                    tpe[0:1, :], kvt2_sb[0:64, kE:kE + 1], ident_sb[0:64, 0:64]
                )
                nc.vector.tensor_copy(kvaug_sb[0:1, cE * 65:cE * 65 + 64],
                                      tpe[0:1, :])
                nc.vector.memset(kvaug_sb[0:1, cE * 65 + 64:cE * 65 + 65], 1.0)
                nc.vector.tensor_copy(
                    kvaug2_sb[0:1, cE * 128 + 64:cE * 128 + 128], tpe[0:1, :])
                nc.vector.memset(
                    kvaug2_sb[0:1, cE * 128 + 32:cE * 128 + 33], 1.0)

            def emit_attn_pair(pair, qb):
                qs = slice(qb * QB, (qb + 1) * QB)
                cE = 4 * qb + 4
                kE = 128 * cE
                ots = {}

                def st_mm(dst, c, j0, parity):
                    p0 = 64 * parity
                    nc.tensor.matmul(
                        dst,
                        lhsT=kvt2_sb[p0:p0 + 64, c * 128:(c + 1) * 128],
                        rhs=qt2_sb[p0:p0 + 64, pair, qb * QB + j0:(qb + 1) * QB],
                        start=True,
                        stop=True,
                    )

                def pv_mm(ot, c, rhs_ap, j0, parity):
                    if parity == 0:
                        lhsT = kvaug_sb[:, c * 65:c * 65 + 65]
                        dst = ot[0:65, j0:512]
                    else:
                        lhsT = kvaug2_sb[:, c * 128:(c + 1) * 128]
                        dst = ot[0:128, j0:512]
                    nc.tensor.matmul(
                        dst,
                        lhsT=lhsT,
                        rhs=rhs_ap,
                        start=(c == 0),
                        stop=False,
                    )

                for parity in range(2):
                    p0 = 64 * parity
                    rows = slice(0, 65) if parity == 0 else slice(0, 128)
                    ot = ot_ps.tile([128, 512], f32, tag="ot")
                    ots[parity] = ot

                    for g in range(qb * 2):
                        st = st_ps.tile([128, 1024], f32, tag="st")
                        for i in range(2):
                            st_mm(st[:, i * 512:(i + 1) * 512], 2 * g + i, 0,
                                  parity)
                        expst = work.tile([128, 1024], f16, tag="expst")
                        nc.scalar.activation(expst, st, EXP)
                        for i in range(2):
                            pv_mm(ot, 2 * g + i, expst[:, i * 512:(i + 1) * 512],
                                  0, parity)

                    for grp in range(2):
                        gw = GW[grp]
                        st = st_ps.tile([128, 1024], f32, tag="st")
                        for t, j0, w, g_, off in DIAG:
                            if g_ != grp:
                                continue
                            st_mm(st[:, off:off + w], 4 * qb + t, j0, parity)
                        if grp == 1:
                            # +1-edge key score rides in this tile's tail col
                            nc.tensor.matmul(
                                st[0:1, EDGE_OFF:EDGE_OFF + 1],
                                lhsT=kvt2_sb[p0:p0 + 64, kE:kE + 1],
                                rhs=qt2_sb[p0:p0 + 64, pair,
                                           qb * QB + 511:qb * QB + 512],
                                start=True,
                                stop=True,
                            )
                            gw += 1
                        expst = work.tile([128, 1024], f16, tag="expst")
                        nc.scalar.activation(expst[:, 0:gw], st[:, 0:gw], EXP)
                        for t, j0, w, g_, off in DIAG:
                            if g_ != grp:
                                continue
                            nc.vector.tensor_mul(
                                expst[:, off:off + w],
                                expst[:, off:off + w],
                                masks_sb[:, t * QB + j0:(t + 1) * QB],
                            )
                            pv_mm(ot, 4 * qb + t, expst[:, off:off + w], j0,
                                  parity)
                        if grp == 1:
                            if parity == 0:
                                lhsT = kvaug_sb[0:1, cE * 65:cE * 65 + 65]
                                dst = ot[0:65, 511:512]
                            else:
                                lhsT = kvaug2_sb[0:1, cE * 128:(cE + 1) * 128]
                                dst = ot[0:128, 511:512]
                            nc.tensor.matmul(
                                dst,
                                lhsT=lhsT,
                                rhs=expst[0:1, EDGE_OFF:EDGE_OFF + 1],
                                start=False,
                                stop=True,
                            )

                for parity in range(2):
                    ot = ots[parity]
                    drow = 64 if parity == 0 else 32
                    p0 = 64 * parity
                    recip = small.tile([1, 512], f16, tag="recip")
                    with nc.allow_low_precision(reason="softmax recip f16"):
                        nc.vector.reciprocal(recip, ot[drow:drow + 1, :])
                    bcast = small.tile([128, 512], f16, tag="bcast")
                    if bcast_via == "pool":
                        nc.gpsimd.partition_broadcast(
                            bcast, recip[0:1, :], channels=128)
                    else:
                        rep = st_ps.tile([128, 1024], f32, tag="st")
                        nc.tensor.matmul(
                            rep[p0:p0 + 64, 0:512],
                            lhsT=ones_sb[0:1, 0:64],
                            rhs=recip[0:1, :],
                            start=True,
                            stop=True,
                        )
                        nc.vector.tensor_copy(bcast[p0:p0 + 64, :],
                                              rep[p0:p0 + 64, 0:512])
                    nc.vector.tensor_mul(
                        at_sb[p0:p0 + 64, pair, qs],
                        ot[p0:p0 + 64, :],
                        bcast[p0:p0 + 64, :],
                    )

            def emit_outproj(m):
                ms = slice(m * 128, (m + 1) * 128)
                for nn in range(2):
                    os_ = slice(nn * 512, (nn + 1) * 512)
                    op = pj_ps.tile([128, 512], f32, tag="pj")
                    for fc in range(4):
                        nc.tensor.matmul(
                            op[:, 0:512],
                            lhsT=at_sb[:, fc, ms],
                            rhs=wout_sb[:, fc, os_],
                            start=(fc == 0),
                            stop=(fc == 3),
                        )
                    ost = ostp.tile([128, 512], f32, tag="ost")
                    nc.vector.tensor_copy(ost, op[:, 0:512])
                    nc.sync.dma_start(out=out_d[ms, os_], in_=ost)

            def _emit_all():
                emit_inputs_weights_early()
                qs0 = slice(0, QB)
                nc.sync.dma_start(out=xt_sb[:, 0:4, qs0], in_=xt_r[:, 0:4, qs0])
                nc.sync.dma_start(out=xt_sb[:, 4:8, qs0], in_=xt_r[:, 4:8, qs0])
                nc.sync.dma_start(out=wq_sb[:, 0:4, :], in_=wq_r[:, 0:4, :])
                nc.sync.dma_start(out=wq_sb[:, 4:8, :], in_=wq_r[:, 4:8, :])
                nc.sync.dma_start(out=ident_sb, in_=ident_d)
                nc.sync.dma_start(out=kvt2_sb[:, 0:1], in_=nullkv2_d)
                emit_proj(0)
                emit_kvaug(0)
                emit_inputs_qb(1)
                emit_inputs_weights_late()
                emit_proj(1)
                emit_kvaug(1)
                for qb in range(NQB):
                    if qb + 2 < NQB:
                        emit_inputs_qb(qb + 2)
                    for pair in range(4):
                        emit_attn_pair(pair, qb)
                        # PE filler work between ACT-bound attention stretches
                        if qb + 2 < NQB:
                            if pair == 0:
                                emit_proj_kv(qb + 2)
                            emit_proj_q(qb + 2, pair)
                        if qb >= 1:
                            emit_outproj(4 * (qb - 1) + pair)
                    if qb + 2 < NQB:
                        emit_kvaug(qb + 2)
                for m in range(12, 16):
                    emit_outproj(m)

            if reps:
                import concourse.mybir as _mybir
                with tc.For_i(0, reps, 1):
                    _emit_all()
            else:
                _emit_all()

    nc.finalize()
    return nc


def _host_prep(x, Wq, Wkv, null_kv, Wout):
    x = np.asarray(x, dtype=np.float32)
    Wq = np.asarray(Wq, dtype=np.float32)
    Wkv = np.asarray(Wkv, dtype=np.float32)
    null_kv = np.asarray(null_kv, dtype=np.float32)
    Wout = np.asarray(Wout, dtype=np.float32)

    scale = DH ** -0.5
    wq_scaled = (Wq * scale).astype(np.float16)
    wkv2 = np.concatenate([Wkv, Wkv], axis=1).astype(np.float16)  # [1024,128]
    nullkv2 = np.concatenate([null_kv, null_kv]).astype(np.float16).reshape(128, 1)
    wout16 = Wout.astype(np.float16)
    ident = np.eye(128, dtype=np.float16)

    # masks[t][i, j] = 1 if j >= i + 128*t - 1  (ST layout: i = k within chunk,
    # j = q within 512 block; delta = 128*t - 1 for diagonal chunk t)
    i_idx = np.arange(128)[:, None]
    j_idx = np.arange(QB)[None, :]
    masks = np.concatenate(
        [(j_idx >= i_idx + 128 * t - 1).astype(np.float16) for t in range(4)], axis=1
    )  # [128, 2048]

    in_maps = []
    for core in range(8):
        b, hg = core // 2, core % 2
        in_maps.append(
            {
                "xt": np.ascontiguousarray(x[b].T).astype(np.float16),
                "wq": wq_scaled[:, hg * 512:(hg + 1) * 512].copy(),
                "wkv2": wkv2,
                "nullkv2": nullkv2,
                "wout": np.ascontiguousarray(wout16[hg * 512:(hg + 1) * 512, :]),
                "masks": masks,
                "ident": ident,
            }
        )
    return in_maps


def kernel(x, Wq, Wkv, null_kv, Wout, _trace=False):
    from concourse import bass_utils

    if "nc" not in _PROGRAM_CACHE:
        _PROGRAM_CACHE["nc"] = _build_program()
    nc = _PROGRAM_CACHE["nc"]

    in_maps = _host_prep(x, Wq, Wkv, null_kv, Wout)
    res = bass_utils.run_bass_kernel_spmd(
        nc, in_maps, core_ids=list(range(8)), trace=_trace
    )
    _PROGRAM_CACHE["last_result"] = res

    outs = [np.asarray(r["out"], dtype=np.float32) for r in res.results]
    full = np.stack([outs[2 * b] + outs[2 * b + 1] for b in range(B)], axis=0)
    return full


# revision 29
# speedup vs baseline: 1.1162x; 1.1162x over previous
"""Trainium2 Bass kernel for MQA causal attention with null token.

Problem (reference.py):
  b=4, n=2048, dim=1024, HEADS=16, DIM_HEAD=64
  q  = (x @ Wq).reshape(b,n,16,64).transpose -> [b,h,n,64] * 64**-0.5
  kv = x @ Wkv -> [b,n,64]; prepend null -> [b,2049,64]
  sim = q @ kv^T  (causal: query i sees kv cols 0..i+1)
  out = softmax(sim) @ kv -> concat heads -> @ Wout

Sharding: 8 cores = batch(4) x head-half(2). Each core handles one batch
element and 8 heads, producing a partial out-projection; host adds the two
half-head partials per batch.

Device algorithm (per core), matmuls f16 inputs w/ fp32 PSUM accumulate,
emission interleaved qb-outer so projection / out-projection matmuls fill
PE gaps during ACT(exp)-bound attention stretches:
  per q-block (512 queries): project QT2 (head pairs on partitions) + KV
  block; build KV_aug chunks ([128,65] parity0 / [128,128] parity1 with
  output partitions 63..127); scores TRANSPOSED (keys on partitions) with
  diagonal chunks narrowed to the causally-visible range; exp on ACT
  (PSUM->SBUF f16, the +1-edge key's score rides in the diagonal tile);
  causal mask multiply on DVE; PV accumulates [*,512] with the softmax
  denominator as an extra output row; normalize = DVE reciprocal + GpSimd
  partition-broadcast + DVE multiply into AT; out-projection per 128-token
  chunk contract AT^T @ Wout -> fp32 -> HBM.
"""

import sys

for _p in ("/opt/trn_rl_repo",):
    if _p not in sys.path:
        sys.path.insert(0, _p)

import numpy as np

HEADS = 16
DH = 64
B = 4
N = 2048
DIM = 1024
NQB = 4          # q blocks of 512 per head
QB = 512
KTOT = N + 1     # 2049 kv positions (null at 0)

_PROGRAM_CACHE = {}


def _build_program(reps=0, expst_bufs=10, ot_bufs=2, pj_bufs=2, st_bufs=2,
                   ost_bufs=4, bcast_via="pool"):
    import concourse.bacc as bacc
    import concourse.tile as tile
    import concourse.mybir as mybir
    import concourse.bass as _bass

    f16 = mybir.dt.float16
    f32 = mybir.dt.float32
    EXP = mybir.ActivationFunctionType.Exp

    nc = bacc.Bacc("TRN2", debug=False, num_devices=8)

    xt_d = nc.dram_tensor("xt", [DIM, N], f16, kind="ExternalInput").ap()
    wq_d = nc.dram_tensor("wq", [DIM, 512], f16, kind="ExternalInput").ap()
    wkv2_d = nc.dram_tensor("wkv2", [DIM, 128], f16, kind="ExternalInput").ap()
    nullkv2_d = nc.dram_tensor("nullkv2", [128, 1], f16, kind="ExternalInput").ap()
    wout_d = nc.dram_tensor("wout", [512, DIM], f16, kind="ExternalInput").ap()
    masks_d = nc.dram_tensor("masks", [128, 1410], f16, kind="ExternalInput").ap()
    ident_d = nc.dram_tensor("ident", [128, 128], f16, kind="ExternalInput").ap()
    out_d = nc.dram_tensor("out", [N, DIM], f32, kind="ExternalOutput").ap()

    # diagonal chunk t: visible cols j >= j0; (j0, width), col offset in tile
    DIAG = []
    off = [0, 0]
    for t in range(4):
        j0 = 0 if t == 0 else 128 * t - 1
        w = QB - j0
        grp = t // 2
        DIAG.append((t, j0, w, grp, off[grp]))
        off[grp] += w
    GW = (off[0], off[1])          # (897, 386)
    EDGE_OFF = off[1]              # edge score column in the grp-1 tile

    with tile.TileContext(nc) as tc:
        from contextlib import ExitStack

        with ExitStack() as ctx:
            consts = ctx.enter_context(tc.tile_pool(name="consts", bufs=1))
            work = ctx.enter_context(tc.tile_pool(name="work", bufs=expst_bufs))
            ostp = ctx.enter_context(tc.tile_pool(name="ostp", bufs=ost_bufs))
            small = ctx.enter_context(tc.tile_pool(name="small", bufs=3))
            # PSUM budget (8 banks): st 2x2 + ot 2x1 + pj 2x1 = 8
            st_ps = ctx.enter_context(tc.tile_pool(
                name="st_ps", bufs=st_bufs, space="PSUM"))
            ot_ps = ctx.enter_context(tc.tile_pool(
                name="ot_ps", bufs=ot_bufs, space="PSUM"))
            pj_ps = ctx.enter_context(tc.tile_pool(
                name="pj_ps", bufs=pj_bufs, space="PSUM"))

            # ---- persistent SBUF tiles ----
            xt_sb = consts.tile([128, 8, N], f16, tag="xt")
            wq_sb = consts.tile([128, 8, 512], f16, tag="wq")
            wkv2_sb = consts.tile([128, 8, 128], f16, tag="wkv2")
            wout_sb = consts.tile([128, 4, DIM], f16, tag="wout")
            masks_sb = consts.tile([128, 1410], f16, tag="masks")
            ident_sb = consts.tile([128, 128], f16, tag="ident")
            kvt2_sb = consts.tile([128, KTOT], f16, tag="kvt2")
            kvaug_sb = consts.tile([128, 17 * 65], f16, tag="kvaug")
            kvaug2_sb = consts.tile([128, 17 * 128], f16, tag="kvaug2")
            qt2_sb = consts.tile([128, 4, N], f16, tag="qt2")
            at_sb = consts.tile([128, 4, N], f16, tag="at")
            ones_sb = None
            if bcast_via == "pe":
                ones_sb = consts.tile([128, 64], f16, tag="ones")

            # one-time setup (outside the timing loop): constant structure of
            # the kv-augmented tiles (ones columns / zero padding)
            nc.vector.memset(kvaug2_sb, 0.0)
            nc.vector.memset(kvaug_sb, 1.0)
            for _c in range(17):
                nc.vector.memset(
                    kvaug2_sb[:, _c * 128 + 32:_c * 128 + 33], 1.0)
            if ones_sb is not None:
                nc.vector.memset(ones_sb, 1.0)

            xt_r = xt_d.rearrange("(d p) t -> p d t", p=128)

            def emit_inputs_qb(qb):
                qs = slice(qb * QB, (qb + 1) * QB)
                nc.sync.dma_start(out=xt_sb[:, :, qs], in_=xt_r[:, :, qs])

            wq_r = wq_d.rearrange("(d p) m -> p d m", p=128)

            def emit_inputs_weights_early():
                nc.sync.dma_start(out=ident_sb, in_=ident_d)
                nc.sync.dma_start(out=kvt2_sb[:, 0:1], in_=nullkv2_d)
                nc.sync.dma_start(
                    out=wkv2_sb, in_=wkv2_d.rearrange("(d p) m -> p d m", p=128))

            def emit_inputs_weights_late():
                nc.sync.dma_start(out=masks_sb, in_=masks_d)
                nc.sync.dma_start(
                    out=wout_sb, in_=wout_d.rearrange("(f p) o -> p f o", p=128))

            def emit_proj_kv(qb):
                qs = slice(qb * QB, (qb + 1) * QB)
                kp = pj_ps.tile([128, 512], f32, tag="pj")
                for d in range(8):
                    nc.tensor.matmul(
                        kp[:, 0:512],
                        lhsT=wkv2_sb[:, d, :],
                        rhs=xt_sb[:, d, qs],
                        start=(d == 0),
                        stop=(d == 7),
                    )
                nc.vector.tensor_copy(kvt2_sb[:, 1 + qb * 512:513 + qb * 512],
                                   kp[:, 0:512])

            def emit_proj_q(qb, pair):
                qs = slice(qb * QB, (qb + 1) * QB)
                qp = pj_ps.tile([128, 512], f32, tag="pj")
                for d in range(8):
                    nc.tensor.matmul(
                        qp[:, 0:512],
                        lhsT=wq_sb[:, d, pair * 128:(pair + 1) * 128],
                        rhs=xt_sb[:, d, qs],
                        start=(d == 0),
                        stop=(d == 7),
                    )
                nc.vector.tensor_copy(qt2_sb[:, pair, qs], qp[:, 0:512])

            def emit_proj(qb):
                emit_proj_kv(qb)
                for pair in range(4):
                    emit_proj_q(qb, pair)

            def emit_kvaug(qb):
                for c in range(4 * qb, 4 * qb + 4):
                    tp = pj_ps.tile([128, 64], f16, tag="pj")
                    nc.tensor.transpose(
                        tp, kvt2_sb[0:64, c * 128:(c + 1) * 128],
                        ident_sb[0:64, 0:64]
                    )
                    nc.vector.tensor_copy(kvaug_sb[:, c * 65:c * 65 + 64], tp)
                    nc.vector.tensor_copy(
                        kvaug2_sb[:, c * 128 + 64:c * 128 + 128], tp)
                cE = 4 * qb + 4
                kE = 128 * cE
                tpe = pj_ps.tile([128, 64], f16, tag="pj")
                nc.tensor.transpose(
                    tpe[0:1, :], kvt2_sb[0:64, kE:kE + 1], ident_sb[0:64, 0:64]
                )
                nc.vector.tensor_copy(kvaug_sb[0:1, cE * 65:cE * 65 + 64],
                                      tpe[0:1, :])
                nc.vector.tensor_copy(
                    kvaug2_sb[0:1, cE * 128 + 64:cE * 128 + 128], tpe[0:1, :])

            def emit_attn_pair(pair, qb):
                qs = slice(qb * QB, (qb + 1) * QB)
                cE = 4 * qb + 4
                kE = 128 * cE
                ots = {}

                def st_mm(dst, c, j0, parity):
                    p0 = 64 * parity
                    nc.tensor.matmul(
                        dst,
                        lhsT=kvt2_sb[p0:p0 + 64, c * 128:(c + 1) * 128],
                        rhs=qt2_sb[p0:p0 + 64, pair, qb * QB + j0:(qb + 1) * QB],
                        start=True,
                        stop=True,
                    )

                def pv_mm(ot, c, rhs_ap, j0, parity):
                    if parity == 0:
                        lhsT = kvaug_sb[:, c * 65:c * 65 + 65]
                        dst = ot[0:65, j0:512]
                    else:
                        lhsT = kvaug2_sb[:, c * 128:(c + 1) * 128]
                        dst = ot[0:128, j0:512]
                    nc.tensor.matmul(
                        dst,
                        lhsT=lhsT,
                        rhs=rhs_ap,
                        start=(c == 0),
                        stop=False,
                    )

                for parity in range(2):
                    p0 = 64 * parity
                    rows = slice(0, 65) if parity == 0 else slice(0, 128)
                    ot = ot_ps.tile([128, 512], f32, tag="ot")
                    ots[parity] = ot

                    for g in range(qb * 2):
                        st = st_ps.tile([128, 1024], f32, tag="st")
                        for i in range(2):
                            st_mm(st[:, i * 512:(i + 1) * 512], 2 * g + i, 0,
                                  parity)
                        expst = work.tile([128, 1024], f16, tag="expst")
                        nc.scalar.activation(expst, st, EXP)
                        for i in range(2):
                            pv_mm(ot, 2 * g + i, expst[:, i * 512:(i + 1) * 512],
                                  0, parity)

                    for grp in range(2):
                        gw = GW[grp]
                        st = st_ps.tile([128, 1024], f32, tag="st")
                        for t, j0, w, g_, off in DIAG:
                            if g_ != grp:
                                continue
                            st_mm(st[:, off:off + w], 4 * qb + t, j0, parity)
                        if grp == 1:
                            # +1-edge key score rides in this tile's tail col
                            nc.tensor.matmul(
                                st[0:1, EDGE_OFF:EDGE_OFF + 1],
                                lhsT=kvt2_sb[p0:p0 + 64, kE:kE + 1],
                                rhs=qt2_sb[p0:p0 + 64, pair,
                                           qb * QB + 511:qb * QB + 512],
                                start=True,
                                stop=True,
                            )
                            gw += 1
                        expst = work.tile([128, 1024], f16, tag="expst")
                        nc.scalar.activation(expst[:, 0:gw], st[:, 0:gw], EXP)
                        moff = 0 if grp == 0 else 1024
                        nc.vector.tensor_mul(
                            expst[:, 0:GW[grp]],
                            expst[:, 0:GW[grp]],
                            masks_sb[:, moff:moff + GW[grp]],
                        )
                        for t, j0, w, g_, off in DIAG:
                            if g_ != grp:
                                continue
                            pv_mm(ot, 4 * qb + t, expst[:, off:off + w], j0,
                                  parity)
                        if grp == 1:
                            if parity == 0:
                                lhsT = kvaug_sb[0:1, cE * 65:cE * 65 + 65]
                                dst = ot[0:65, 511:512]
                            else:
                                lhsT = kvaug2_sb[0:1, cE * 128:(cE + 1) * 128]
                                dst = ot[0:128, 511:512]
                            nc.tensor.matmul(
                                dst,
                                lhsT=lhsT,
                                rhs=expst[0:1, EDGE_OFF:EDGE_OFF + 1],
                                start=False,
                                stop=True,
                            )

                for parity in range(2):
                    ot = ots[parity]
                    drow = 64 if parity == 0 else 32
                    p0 = 64 * parity
                    recip = small.tile([1, 512], f16, tag="recip")
                    with nc.allow_low_precision(reason="softmax recip f16"):
                        nc.vector.reciprocal(recip, ot[drow:drow + 1, :])
                    bcast = small.tile([128, 512], f16, tag="bcast")
                    if bcast_via == "pool":
                        nc.gpsimd.partition_broadcast(
                            bcast, recip[0:1, :], channels=128)
                    else:
                        rep = pj_ps.tile([128, 512], f32, tag="pj")
                        nc.tensor.matmul(
                            rep[p0:p0 + 64, 0:512],
                            lhsT=ones_sb[0:1, 0:64],
                            rhs=recip[0:1, :],
                            start=True,
                            stop=True,
                        )
                        nc.vector.tensor_copy(bcast[p0:p0 + 64, :],
                                              rep[p0:p0 + 64, :])
                    nc.vector.tensor_mul(
                        at_sb[p0:p0 + 64, pair, qs],
                        ot[p0:p0 + 64, :],
                        bcast[p0:p0 + 64, :],
                    )

            def emit_outproj(m, split=False):
                ms = slice(m * 128, (m + 1) * 128)
                for nn in range(2):
                    os_ = slice(nn * 512, (nn + 1) * 512)
                    op = pj_ps.tile([128, 512], f32, tag="pj")
                    if split:
                        for par in range(2):
                            p0 = 64 * par
                            for fc in range(4):
                                nc.tensor.matmul(
                                    op[:, 0:512],
                                    lhsT=at_sb[p0:p0 + 64, fc, ms],
                                    rhs=wout_sb[p0:p0 + 64, fc, os_],
                                    start=(par == 0 and fc == 0),
                                    stop=(par == 1 and fc == 3),
                                )
                    else:
                        for fc in range(4):
                            nc.tensor.matmul(
                                op[:, 0:512],
                                lhsT=at_sb[:, fc, ms],
                                rhs=wout_sb[:, fc, os_],
                                start=(fc == 0),
                                stop=(fc == 3),
                            )
                    ost = ostp.tile([128, 512], f32, tag="ost")
                    nc.vector.tensor_copy(ost, op[:, 0:512])
                    nc.sync.dma_start(out=out_d[ms, os_], in_=ost)

            def _emit_all():
                emit_inputs_weights_early()
                qs0 = slice(0, QB)
                nc.sync.dma_start(out=xt_sb[:, 0:4, qs0], in_=xt_r[:, 0:4, qs0])
                nc.sync.dma_start(out=xt_sb[:, 4:8, qs0], in_=xt_r[:, 4:8, qs0])
                nc.sync.dma_start(out=wq_sb[:, 0:4, :], in_=wq_r[:, 0:4, :])
                nc.sync.dma_start(out=wq_sb[:, 4:8, :], in_=wq_r[:, 4:8, :])
                emit_proj(0)
                emit_kvaug(0)
                emit_inputs_weights_late()
                for qb in range(NQB):
                    if qb + 1 < NQB:
                        emit_inputs_qb(qb + 1)
                    for pair in range(4):
                        emit_attn_pair(pair, qb)
                        # PE filler work between ACT-bound attention stretches
                        if qb + 1 < NQB:
                            if pair == 0:
                                emit_proj_kv(qb + 1)
                            emit_proj_q(qb + 1, pair)
                        if qb >= 1:
                            emit_outproj(4 * (qb - 1) + pair)
                    if qb + 1 < NQB:
                        emit_kvaug(qb + 1)
                for m in range(12, 16):
                    emit_outproj(m)

            if reps:
                hints = (mybir.EngineType.PE, mybir.EngineType.Activation,
                         mybir.EngineType.DVE, mybir.EngineType.Pool,
                         mybir.EngineType.SP)
                with tc.For_i(0, reps, 1, hint_engines=hints,
                              staggered_reset=True):
                    _emit_all()
            else:
                _emit_all()

    nc.finalize()
    return nc


def _host_prep(x, Wq, Wkv, null_kv, Wout):
    x = np.asarray(x, dtype=np.float32)
    Wq = np.asarray(Wq, dtype=np.float32)
    Wkv = np.asarray(Wkv, dtype=np.float32)
    null_kv = np.asarray(null_kv, dtype=np.float32)
    Wout = np.asarray(Wout, dtype=np.float32)

    scale = DH ** -0.5
    wq_scaled = (Wq * scale).astype(np.float16)
    wkv2 = np.concatenate([Wkv, Wkv], axis=1).astype(np.float16)  # [1024,128]
    nullkv2 = np.concatenate([null_kv, null_kv]).astype(np.float16).reshape(128, 1)
    wout16 = Wout.astype(np.float16)
    ident = np.eye(128, dtype=np.float16)

    # masks[t][i, j] = 1 if j >= i + 128*t - 1  (ST layout: i = k within chunk,
    # j = q within 512 block; delta = 128*t - 1 for diagonal chunk t)
    i_idx = np.arange(128)[:, None]
    j_idx = np.arange(QB)[None, :]
    mk = [(j_idx >= i_idx + 128 * t - 1).astype(np.float16) for t in range(4)]
    # contiguous per-diag-group layout: grp0 = [t0 | t1[:,127:]], grp1 =
    # [t2[:,255:] | t3[:,383:]] matching the score-tile column packing
    pad = np.zeros((128, 1024 - 897), np.float16)
    masks = np.concatenate(
        [mk[0], mk[1][:, 127:], pad, mk[2][:, 255:], mk[3][:, 383:]], axis=1
    )  # [128, 1410]: grp0 at 0 (897), grp1 at 1024 (386, 4B-aligned)

    in_maps = []
    for core in range(8):
        b, hg = core // 2, core % 2
        in_maps.append(
            {
                "xt": np.ascontiguousarray(x[b].T).astype(np.float16),
                "wq": wq_scaled[:, hg * 512:(hg + 1) * 512].copy(),
                "wkv2": wkv2,
                "nullkv2": nullkv2,
                "wout": np.ascontiguousarray(wout16[hg * 512:(hg + 1) * 512, :]),
                "masks": masks,
                "ident": ident,
            }
        )
    return in_maps


def kernel(x, Wq, Wkv, null_kv, Wout, _trace=False):
    from concourse import bass_utils

    if "nc" not in _PROGRAM_CACHE:
        _PROGRAM_CACHE["nc"] = _build_program()
    nc = _PROGRAM_CACHE["nc"]

    in_maps = _host_prep(x, Wq, Wkv, null_kv, Wout)
    res = bass_utils.run_bass_kernel_spmd(
        nc, in_maps, core_ids=list(range(8)), trace=_trace
    )
    _PROGRAM_CACHE["last_result"] = res

    outs = [np.asarray(r["out"], dtype=np.float32) for r in res.results]
    full = np.stack([outs[2 * b] + outs[2 * b + 1] for b in range(B)], axis=0)
    return full
